# revision 5
# baseline (speedup 1.0000x reference)
"""Multi-head causal attention (B=2, S=2048, D=1024, H=16) on 8 trn2 cores.

Sharding: core c -> batch b=c//4, head-group g=c%4 (heads 4g..4g+3).
Each core: Q/K/V projections for its heads from xT[b], causal attention in
transposed layout, row-parallel out-projection partial. Host sums the 4
partials per batch and adds the output bias.

Schedule notes (v2):
- PE pre-warm: dummy matmuls on junk SBUF during the input-DMA lead-in so
  the HAM clock gate is at 8/8 when real matmuls start.
- Attention emits scores one k-group ahead of the PV matmuls (both heads
  interleaved) so the PE never waits on ACT's exp latency.
- Score/exp/PV ranges are trimmed to the causally-live columns; dead
  columns are never consumed, so no masking memsets are needed.
- qk pair-1 projection chunks fill PE slack inside pair-0 attention;
  out-projection chunks fill pair-1 attention.
- 1/l via reciprocal_approx_fast (single DVE op) instead of the iterative
  reciprocal (3.4us -> 0.7us per call).
"""

import collections

import numpy as np

import concourse.bass as bass
import concourse.tile as tile
import concourse.mybir as mybir
from concourse import bacc
from concourse.bass_utils import run_bass_kernel_spmd

B, S, D, H, DH = 2, 2048, 1024, 16, 64
NCORES = 8
HPC = 4          # heads per core
PAIRS = 2        # head pairs per core
QT = 512         # q tile (free dim of scoresT / PV matmuls)
KB = 128         # k block (partition dim of scoresT)
NQT = S // QT    # 4
NKB = S // KB    # 16
DC = D // 128    # 8 contraction chunks for projections
SCALE = 1.0 / np.sqrt(DH)

F32 = mybir.dt.float32
BF = mybir.dt.bfloat16


def _build():
    nc = bacc.Bacc("TRN2", target_bir_lowering=False, debug=False, num_devices=NCORES)

    xT = nc.dram_tensor("xT", [D, S], BF, kind="ExternalInput").ap()
    wq = nc.dram_tensor("wq", [D, HPC * DH], BF, kind="ExternalInput").ap()
    wk = nc.dram_tensor("wk", [D, HPC * DH], BF, kind="ExternalInput").ap()
    wv = nc.dram_tensor("wv", [D, HPC * DH], BF, kind="ExternalInput").ap()
    wo = nc.dram_tensor("wo", [HPC * DH, D], BF, kind="ExternalInput").ap()
    tri = nc.dram_tensor("tri", [KB, KB], BF, kind="ExternalInput").ap()
    out = nc.dram_tensor("out", [S, D], F32, kind="ExternalOutput").ap()

    with tile.TileContext(nc) as tc, \
         tc.tile_pool(name="persist", bufs=1) as persist:
        # ---- persistent tiles ----
        qt_sb = [persist.tile([128, S], BF, name=f"qt{p}", tag=f"qt{p}") for p in range(PAIRS)]
        kt_sb = [persist.tile([128, S], BF, name=f"kt{p}", tag=f"kt{p}") for p in range(PAIRS)]
        # V' tiles: per s-block j, [128, 4*65]; head hl at cols 65*hl, ones col at 65*hl+64
        vt_sb = [persist.tile([128, HPC * (DH + 1)], BF, name=f"vt{j}", tag=f"vt{j}") for j in range(NKB)]
        ctx_sb = [persist.tile([128, S], BF, name=f"ctx{p}", tag=f"ctx{p}") for p in range(PAIRS)]
        wo_sb = [persist.tile([128, D], BF, name=f"wo{p}", tag=f"wo{p}") for p in range(PAIRS)]
        tri_sb = persist.tile([KB, KB], BF, name="tri", tag="tri")
        junk = persist.tile([128, QT], BF, name="junk", tag="junk")  # never written: PE warmup fuel

        xts = [persist.tile([128, S], BF, name=f"xts{i}", tag=f"xts{i}") for i in range(DC)]
        wq_sb = [persist.tile([128, HPC * DH], BF, name=f"wq{i}", tag=f"wq{i}") for i in range(DC)]
        wk_sb = [persist.tile([128, HPC * DH], BF, name=f"wk{i}", tag=f"wk{i}") for i in range(DC)]
        wv_sb = [persist.tile([128, HPC * DH], BF, name=f"wv{i}", tag=f"wv{i}") for i in range(DC)]

        nc.sync.dma_start(tri_sb[:], tri[:])
        for i in range(DC):
            nc.sync.dma_start(xts[i][:], xT[i * 128:(i + 1) * 128, :])
            nc.sync.dma_start(wq_sb[i][:], wq[i * 128:(i + 1) * 128, :])
            nc.sync.dma_start(wk_sb[i][:], wk[i * 128:(i + 1) * 128, :])
        for i in range(DC):
            nc.sync.dma_start(wv_sb[i][:], wv[i * 128:(i + 1) * 128, :])
        for p in range(PAIRS):
            nc.sync.dma_start(wo_sb[p][:], wo[p * 128:(p + 1) * 128, :])

        # ---- PE warm-up: ~3.5us of junk matmuls while input DMAs land ----
        nc.gpsimd.memset(junk[:], 0.0)
        with tc.tile_pool(name="warm", bufs=1, space="PSUM") as wps:
            wt = wps.tile([128, QT], F32, name="warm", tag="warm")
            for _ in range(18):
                nc.tensor.matmul(wt[:], junk[:, 0:128], junk[:], start=True, stop=True)

        def proj_qk_chunked(p, pool):
            """q/k projection for pair p, D-chunk-outer so matmuls chase the
            xT DMAs chunk by chunk. Holds 8 psum banks."""
            qps = [pool.tile([128, QT], F32, name=f"qps{st}", tag=f"qk{st}") for st in range(NQT)]
            kps = [pool.tile([128, QT], F32, name=f"kps{st}", tag=f"qk{4 + st}") for st in range(NQT)]
            for i in range(DC):
                for st in range(NQT):
                    nc.tensor.matmul(
                        qps[st][:], wq_sb[i][:, p * 128:(p + 1) * 128],
                        xts[i][:, st * QT:(st + 1) * QT],
                        start=(i == 0), stop=(i == DC - 1))
                for st in range(NQT):
                    nc.tensor.matmul(
                        kps[st][:], wk_sb[i][:, p * 128:(p + 1) * 128],
                        xts[i][:, st * QT:(st + 1) * QT],
                        start=(i == 0), stop=(i == DC - 1))
            for st in range(NQT):
                nc.scalar.copy(qt_sb[p][:, st * QT:(st + 1) * QT], qps[st][:])
                nc.vector.tensor_copy(kt_sb[p][:, st * QT:(st + 1) * QT], kps[st][:])

        def qk1_chunks(pool):
            """pair-1 q/k projection as 8 filler chunks (one [128,512] psum
            tile each: 8 matmuls + a DVE evac)."""
            def mk(which, st):
                def emit():
                    pp = pool.tile([128, QT], F32, name="qk1", tag="qkseq")
                    w = wq_sb if which == 0 else wk_sb
                    dst = qt_sb[1] if which == 0 else kt_sb[1]
                    for i in range(DC):
                        nc.tensor.matmul(
                            pp[:], w[i][:, 128:256],
                            xts[i][:, st * QT:(st + 1) * QT],
                            start=(i == 0), stop=(i == DC - 1))
                    nc.vector.tensor_copy(dst[:, st * QT:(st + 1) * QT], pp[:])
                return emit
            return [mk(w, st) for st in range(NQT) for w in range(2)]

        def out_chunks(qt_i, ph3ps, ph3sb):
            """partial out-projection for one q tile as 8 filler chunks
            (qb x nh). Bias is applied on the host."""
            chunks = []
            for qb in range(qt_i * 4, qt_i * 4 + 4):
                osref = {}
                def mk(qb, nh, osref):
                    def emit():
                        if nh == 0:
                            osref['t'] = ph3sb.tile([128, D], F32, name="os", tag="os")
                        os_ = osref['t']
                        op = ph3ps.tile([128, 512], F32, name="op", tag="op")
                        for p in range(PAIRS):
                            nc.tensor.matmul(
                                op[:], ctx_sb[p][:, qb * 128:(qb + 1) * 128],
                                wo_sb[p][:, nh * 512:(nh + 1) * 512],
                                start=(p == 0), stop=(p == PAIRS - 1))
                        nc.vector.tensor_copy(os_[:, nh * 512:(nh + 1) * 512], op[:])
                        if nh == 1:
                            nc.sync.dma_start(out[qb * 128:(qb + 1) * 128, :], os_[:])
                    return emit
                for nh in range(2):
                    chunks.append(mk(qb, nh, osref))
            return chunks

        def attention_qt(p, qt_i, scps, ctxps, att, attsm, fillers, per_slot):
            """Causal attention for both heads of pair p on q tile qt_i.

            Emission order per k-group slot: scores+exp for slot i+1 (both
            heads), then `per_slot` filler chunks, then PV for slot i. The
            one-group score lookahead keeps the PE busy while ACT runs exp.
            Score/exp/PV ranges are trimmed to causally-live columns; dead
            columns hold garbage that nothing consumes.
            """
            q0 = qt_i * QT
            nkb = 4 * (qt_i + 1)
            gs = list(range(0, nkb, 2))
            cps = [ctxps.tile([DH + 1, QT], F32, name=f"cps{h}", tag=f"cps{h}")
                   for h in range(2)]
            pts = {}

            def live0(kb):
                d = kb - 4 * qt_i
                return KB * d if d > 0 else 0

            def emit_scores(g0):
                for h in range(2):
                    hl = 2 * p + h
                    r0, r1 = h * 64, h * 64 + 64
                    sp = scps.tile([128, 2 * QT], F32, name="sp", tag="sp")
                    for u in range(2):
                        kb = g0 + u
                        c0 = live0(kb)
                        nc.tensor.matmul(
                            sp[:, u * QT + c0:(u + 1) * QT],
                            kt_sb[p][r0:r1, kb * KB:(kb + 1) * KB],
                            qt_sb[p][r0:r1, q0 + c0:q0 + QT],
                            start=True, stop=True)
                    c0g = live0(g0)
                    pt = att.tile([128, 2 * QT], BF, name="pt", tag="pt")
                    nc.scalar.activation(
                        pt[:, c0g:2 * QT], sp[:, c0g:2 * QT],
                        mybir.ActivationFunctionType.Exp, scale=float(SCALE))
                    # zero the diagonal triangles (the only dead cells inside
                    # the live ranges)
                    if g0 == 4 * qt_i:          # blocks d=0, d=1
                        for off in (0, QT + KB):
                            nc.vector.tensor_mul(
                                pt[:, off:off + KB], pt[:, off:off + KB], tri_sb[:])
                    elif g0 == 4 * qt_i + 2:    # blocks d=2, d=3
                        for off in (2 * KB, QT + 3 * KB):
                            nc.vector.tensor_mul(
                                pt[:, off:off + KB], pt[:, off:off + KB], tri_sb[:])
                    pts[(h, g0)] = pt

            def emit_pv(g0):
                for h in range(2):
                    hl = 2 * p + h
                    pt = pts.pop((h, g0))
                    for u in range(2):
                        kb = g0 + u
                        c0 = live0(kb)
                        nc.tensor.matmul(
                            cps[h][:, c0:QT],
                            vt_sb[kb][:, hl * (DH + 1):(hl + 1) * (DH + 1)],
                            pt[:, u * QT + c0:(u + 1) * QT],
                            start=(kb == 0), stop=(kb == nkb - 1))

            emit_scores(gs[0])
            for i, g0 in enumerate(gs):
                if i + 1 < len(gs):
                    emit_scores(gs[i + 1])
                for _ in range(per_slot):
                    if fillers:
                        fillers.popleft()()
                emit_pv(g0)

            # normalize: ctx = cps[0:64] * (1/l) with l = cps row 64
            for h in range(2):
                r0 = h * 64
                # custom-DVE ops misread PSUM at a partition offset: stage the
                # l row through SBUF before the approx reciprocal.
                l_sb = attsm.tile([1, QT], F32, name="l_sb", tag="l")
                nc.vector.tensor_copy(l_sb[:], cps[h][DH:DH + 1, :])
                r_sb = attsm.tile([1, QT], F32, name="r_sb", tag="r")
                nc.vector.reciprocal_approx_fast(out=r_sb[:], in_=l_sb[:])
                rb = attsm.tile([64, QT], F32, name="rb", tag="rb")
                nc.gpsimd.partition_broadcast(rb[:], r_sb[:])
                nc.vector.tensor_mul(
                    ctx_sb[p][r0:r0 + 64, q0:q0 + QT], cps[h][0:DH, :], rb[:])

        # phase A: q/k pair 0, chunk-pipelined against the input DMAs
        with tc.tile_pool(name="qk0ps", bufs=1, space="PSUM") as qk0ps:
            proj_qk_chunked(0, qk0ps)

        # phase B onwards
        with tc.tile_pool(name="att", bufs=4) as att, \
             tc.tile_pool(name="attsm", bufs=2) as attsm, \
             tc.tile_pool(name="scps", bufs=2, space="PSUM") as scps, \
             tc.tile_pool(name="ctxps", bufs=1, space="PSUM") as ctxps:

            # V projection (dense PE, 2 psum banks), evac to bf16 V' tiles
            with tc.tile_pool(name="vps", bufs=2, space="PSUM") as vps:
                for j in range(NKB):
                    vp = vps.tile([128, HPC * DH], F32, name="vp", tag="vp")
                    for i in range(DC):
                        nc.tensor.matmul(
                            vp[:], xts[i][:, j * 128:(j + 1) * 128], wv_sb[i][:],
                            start=(i == 0), stop=(i == DC - 1))
                    vt_view = vt_sb[j].rearrange("p (h e) -> p h e", h=HPC)
                    nc.vector.tensor_copy(
                        vt_view[:, :, 0:DH], vp.rearrange("p (h e) -> p h e", h=HPC))
                    nc.gpsimd.memset(vt_view[:, :, DH:DH + 1], 1.0)

            # pair-0 attention with pair-1 q/k projection chunks as PE fillers
            with tc.tile_pool(name="qk1ps", bufs=2, space="PSUM") as qk1ps:
                fillers = collections.deque(qk1_chunks(qk1ps))
                for qt_i in range(NQT):
                    attention_qt(0, qt_i, scps, ctxps, att, attsm, fillers, 1)
                while fillers:
                    fillers.popleft()()

            # pair-1 attention with out-projection chunks as PE fillers
            with tc.tile_pool(name="ph3ps", bufs=2, space="PSUM") as ph3ps, \
                 tc.tile_pool(name="ph3sb", bufs=3) as ph3sb:
                fillers = collections.deque()
                for qt_i in range(NQT):
                    attention_qt(1, qt_i, scps, ctxps, att, attsm, fillers, 2)
                    fillers.extend(out_chunks(qt_i, ph3ps, ph3sb))
                while fillers:
                    fillers.popleft()()

    nc.compile()
    return nc


_NC = None
PROFILE = False
TRACE_CORES = (0,)
LAST_RESULT = None


def _get_nc():
    global _NC
    if _NC is None:
        _NC = _build()
    return _NC


def kernel(x, Wq, Wk, Wv, Wo, bo):
    x = np.asarray(x, dtype=np.float32)
    Wq = np.asarray(Wq, dtype=np.float32)
    Wk = np.asarray(Wk, dtype=np.float32)
    Wv = np.asarray(Wv, dtype=np.float32)
    Wo = np.asarray(Wo, dtype=np.float32)
    bo = np.asarray(bo, dtype=np.float32)

    nc = _get_nc()

    in_maps = _prepare_in_maps(x, Wq, Wk, Wv, Wo)

    global LAST_RESULT
    kw = {}
    if PROFILE:
        kw = dict(trace=True, trace_cores=list(TRACE_CORES))
    res = run_bass_kernel_spmd(nc, in_maps, core_ids=list(range(NCORES)), **kw)
    LAST_RESULT = res

    out = np.zeros((B, S, D), np.float32)
    for c in range(NCORES):
        b = c // 4
        out[b] += res.results[c]["out"]
    out += bo.astype(np.float32)
    return out


def _prepare_in_maps(x, Wq, Wk, Wv, Wo):
    kk = np.arange(KB)[:, None]
    qq = np.arange(KB)[None, :]
    import ml_dtypes
    bf16 = ml_dtypes.bfloat16
    tri = (kk <= qq).astype(bf16)

    xTs = [np.ascontiguousarray(x[b].T).astype(bf16) for b in range(B)]

    in_maps = []
    for c in range(NCORES):
        b, g = divmod(c, 4)
        cs = slice(g * HPC * DH, (g + 1) * HPC * DH)
        in_maps.append({
            "xT": xTs[b],
            "wq": np.ascontiguousarray(Wq[:, cs]).astype(bf16),
            "wk": np.ascontiguousarray(Wk[:, cs]).astype(bf16),
            "wv": np.ascontiguousarray(Wv[:, cs]).astype(bf16),
            "wo": np.ascontiguousarray(Wo[cs, :]).astype(bf16),
            "tri": tri,
        })
    return in_maps
